# revision 36
# baseline (speedup 1.0000x reference)
"""AdaptiveCoverageAttention TRN2 kernel: 8-way (batch x head-group) sharded.

Sharding: core c in 0..7 -> batch b = c//4, head-group hg = c%4 (4 heads each).
Each core computes its 4 heads' attention + its partial output projection;
the host sums the 4 partials per batch (and adds b_out). No collectives.

v5: PE-roofline oriented (PE ~393k cycles/core @2.4GHz = 164us).
- IC_W=512: every stream PSUM tile is one bank. pss bufs=3 gives the
  S->exp->S chain 1.5 iterations of slack; po bufs=4 gives normalize a
  whole block of slack. Job pool (1 bank) hosts all projection/MLP/out-proj
  matmuls INTERLEAVED into the stream so the in-order PE queue never
  drains (keeps the PE DVFS p-state at 2.4GHz).
- exp tiles [128,512]: hh1/jt-odd quarter runs on VectorE via Schraudolph
  bf16 (int16 convert + bitcast, mean-centered C=-7.37, ~+7e-3 rel err),
  rest on ScalarE.
- Pooled sums for the gate MLP: half on DVE reduce, half via ScalarE
  activation accum_out, so the gate (which gates the first exp) is ready
  ~23us in.
- Host pre-packs everything partition-major; ~17 large DMAs on the two
  HW DGE rings, small consts packed into 3 DMAs.
- Normalize per (p,ic): dd copy + reciprocal on DVE (recip misreads
  partition-offset PSUM APs, so copy to partition 0 first), broadcast on
  GpSimd, mul on DVE.
- Out-projection runs as jobs after both pairs of an i-range normalize;
  last block's 4 tiles in a short tail.
"""
import os as _os
import sys

sys.path.insert(0, "/opt/trn_rl_repo")

import numpy as np

B, N, D, H = 2, 2048, 1024, 16
HD = D // H            # 64
NCORES = 8
IC_W = 512

_COMPILED = {}

SCHRAUD_A = float(128.0 * np.log2(np.e))
SCHRAUD_B = float(127.0 * 128.0 - 7.37)
_DVE_OFF = bool(int(_os.environ.get("KDVE_OFF", "0")))


def _bf16(x):
    import ml_dtypes
    return np.ascontiguousarray(np.asarray(x, np.float32)).astype(ml_dtypes.bfloat16)


def _dve_tile(jt, hh):
    """Which exp tiles run on VectorE (Schraudolph). 50% of tiles: the
    hh1 tiles, so the hh0 PSUM-bank chain runs through ScalarE and the
    hh1 chain through VectorE, fully decoupled."""
    if _DVE_OFF:
        return False
    return hh == 1


def build(n=N):
    import contextlib

    import concourse.bacc as bacc
    import concourse.tile as tile
    from concourse import mybir

    f32 = mybir.dt.float32
    bf16 = mybir.dt.bfloat16
    i16 = mybir.dt.int16
    AFT = mybir.ActivationFunctionType
    ALU = mybir.AluOpType

    NJ = n // 128          # 16 j-tiles
    NI = n // 512          # 4 i-chunks of 512 (also = stream blocks/pair)
    DC = D // 128          # 8 contraction chunks
    scale = float(HD) ** -0.5

    nc = bacc.Bacc("TRN2", target_bir_lowering=False, debug=False,
                   num_devices=NCORES)

    dram = lambda name, shape, dt, kind: nc.dram_tensor(name, shape, dt, kind=kind).ap()
    XT = dram("xT", [128, 2, DC, 1024], bf16, "ExternalInput")     # (p, jc2, dc, tok)
    WQK = dram("wqk", [128, 4, DC, 128], bf16, "ExternalInput")    # (p, cb, dc, col)
    WV = dram("wv", [128, DC, 256], bf16, "ExternalInput")
    WO = dram("wo", [128, 2, D], bf16, "ExternalInput")
    CVW = dram("cvw", [1, n + 256], bf16, "ExternalInput")         # covT | wce1
    WCE2 = dram("wce2", [128, 8], bf16, "ExternalInput")
    SMF = dram("smf", [128, 11], f32, "ExternalInput")  # bce1|bce2|bfg1|wfg2|bfg2
    WFG1 = dram("wfg1", [128, DC, 256], bf16, "ExternalInput")
    OUT = dram("out", [n, D], f32, "ExternalOutput")
    dbg = bool(int(_os.environ.get("KDBG", "0")))
    if dbg:
        DBG_Q = dram("dbg_q", [128, 2, n], bf16, "ExternalOutput")
        DBG_K = dram("dbg_k", [128, 2, n], bf16, "ExternalOutput")
        DBG_V = dram("dbg_v", [128, NJ, 4, 65], bf16, "ExternalOutput")
        DBG_B = dram("dbg_b", [128, NJ, 4], f32, "ExternalOutput")
        DBG_A = dram("dbg_a", [128, 2, n], bf16, "ExternalOutput")

    with tile.TileContext(nc) as tc, contextlib.ExitStack() as ctx:
        consts = ctx.enter_context(tc.tile_pool(name="consts", bufs=1))
        xtp = ctx.enter_context(tc.tile_pool(name="xtp", bufs=1))
        qkv = ctx.enter_context(tc.tile_pool(name="qkv", bufs=1))
        big2 = ctx.enter_context(tc.tile_pool(name="big2", bufs=1))
        ep = ctx.enter_context(tc.tile_pool(name="ep", bufs=8))
        rp = ctx.enter_context(tc.tile_pool(name="rp", bufs=4))
        yp = ctx.enter_context(tc.tile_pool(name="yp", bufs=6))

        wqk_sb = consts.tile([128, 4, DC, 128], bf16)
        wv_sb = consts.tile([128, DC, 256], bf16)
        wo_sb = consts.tile([128, 2, D], bf16)
        cvw_sb = consts.tile([1, n + 256], bf16)
        covT_sb = cvw_sb[:, 0:n]
        wce1_sb = cvw_sb[:, n:n + 256]
        wce2_sb = consts.tile([128, 8], bf16)
        smf_sb = consts.tile([128, 11], f32)
        bce1_sb = smf_sb[:, 0:2]
        bce2_sb = smf_sb[:, 2:6]
        bfg1_sb = smf_sb[:, 6:8]
        wfg2_sb = smf_sb[:, 8:10]
        bfg2_sb = smf_sb[0:1, 10:11]
        wfg1_sb = consts.tile([128, DC, 256], bf16)
        xts = xtp.tile([128, 2, DC, 1024], bf16, name="xts")

        ones_f = consts.tile([1, 128], f32)
        pooled4 = consts.tile([128, DC, 2], f32)
        pooled_sb = consts.tile([128, DC], f32)
        pooled_bf = consts.tile([128, DC], bf16)
        trash = consts.tile([128, 1024], f32)
        hidg_sb = consts.tile([128, 2], f32)
        g_sb = consts.tile([1, 1], f32)
        gb_sb = consts.tile([128, 1], f32)
        bias_sb = consts.tile([128, NJ, 4], f32)
        bias_dve = consts.tile([128, NJ, 4], f32)

        # ---- DMA schedule: two HW rings, time-critical first. The first
        # 512 tokens of jc0 go in their own (smaller) DMAs so the first
        # qk chunk can start ~2us earlier. Pooled partial sums ride along:
        # DVE reduces the lo halves, ScalarE accum_out the hi halves
        # (emitted between this ring's own DMA issues).
        nc.sync.dma_start(out=wqk_sb[:, 2], in_=WQK[:, 2])       # K0
        nc.sync.dma_start(out=xts[:, 0, 0:4, 0:512], in_=XT[:, 0, 0:4, 0:512])
        nc.sync.dma_start(out=xts[:, 0, 0:4, 512:1024],
                          in_=XT[:, 0, 0:4, 512:1024])
        nc.sync.dma_start(out=wqk_sb[:, 0], in_=WQK[:, 0])       # Q0
        nc.sync.dma_start(out=xts[:, 1, 0:4], in_=XT[:, 1, 0:4])
        nc.sync.dma_start(out=wfg1_sb, in_=WFG1)
        nc.sync.dma_start(out=wo_sb, in_=WO)

        nc.scalar.dma_start(out=xts[:, 0, 4:8, 0:512], in_=XT[:, 0, 4:8, 0:512])
        nc.scalar.dma_start(out=xts[:, 0, 4:8, 512:1024],
                            in_=XT[:, 0, 4:8, 512:1024])
        nc.scalar.dma_start(out=cvw_sb, in_=CVW)
        nc.scalar.dma_start(out=smf_sb, in_=SMF)
        nc.scalar.dma_start(out=wce2_sb, in_=WCE2)
        nc.scalar.dma_start(out=wv_sb, in_=WV)
        nc.scalar.dma_start(out=wqk_sb[:, 3], in_=WQK[:, 3])     # K1
        nc.scalar.dma_start(out=wqk_sb[:, 1], in_=WQK[:, 1])     # Q1
        nc.scalar.dma_start(out=xts[:, 1, 4:8], in_=XT[:, 1, 4:8])

        nc.vector.memset(ones_f, 1.0)
        # 12 of the 16 pooled partials on DVE (jc1-hi comes via ScalarE
        # accum_out, emitted later in the pre-stream so it doesn't block
        # the cov-MLP silus behind late DMA data)
        for jc2 in range(2):
            for dc in range(4):
                nc.vector.reduce_sum(pooled4[:, dc, jc2:jc2 + 1],
                                     xts[:, jc2, dc, :],
                                     axis=mybir.AxisListType.X)
        for dc in range(4, DC):
            nc.vector.reduce_sum(pooled4[:, dc, 0:1], xts[:, 0, dc, :],
                                 axis=mybir.AxisListType.X)

        qt_sb = qkv.tile([128, 2, n], bf16)
        ktp_sb = qkv.tile([128, 2, n], bf16)
        vaug_sb = qkv.tile([128, NJ, 4, 65], bf16)
        nc.vector.memset(vaug_sb, 1.0)
        hidc_sb = big2.tile([128, 2, n], bf16, tag="big", name="hidc")
        attn_sb = big2.tile([128, 2, n], bf16, tag="big", name="attn")

        # ================= stream with interleaved jobs =================
        with tc.tile_pool(name="pss", bufs=4, space="PSUM") as pss, \
             tc.tile_pool(name="pop", bufs=3, space="PSUM") as pop, \
             tc.tile_pool(name="pj", bufs=1, space="PSUM") as pj:

            cp_i = [0]

            def cp_eng():
                cp_i[0] += 1
                return nc.vector if cp_i[0] % 2 == 0 else nc.scalar

            def copy(eng, dst, src):
                if eng is nc.scalar:
                    eng.copy(dst, src)
                else:
                    eng.tensor_copy(dst, src)

            def qk_job(cb, ic):
                pq = pj.tile([128, 512], f32, tag="job", name=f"pq{cb}_{ic}")
                jc2, sub = ic // 2, (ic % 2) * 512
                for dc in range(DC):
                    nc.tensor.matmul(pq, wqk_sb[:, cb, dc, :],
                                     xts[:, jc2, dc, sub:sub + 512],
                                     start=(dc == 0), stop=(dc == DC - 1))
                dst = (ktp_sb[:, cb - 2, ic * 512:(ic + 1) * 512] if cb >= 2
                       else qt_sb[:, cb, ic * 512:(ic + 1) * 512])
                copy(cp_eng(), dst, pq)

            def v_job(it):
                pv = pj.tile([128, 4, 64], f32, tag="job", name=f"pv{it}")
                jc2, col = it // 8, (it % 8) * 128
                for dc in range(DC):
                    nc.tensor.matmul(pv, xts[:, jc2, dc, col:col + 128],
                                     wv_sb[:, dc, :],
                                     start=(dc == 0), stop=(dc == DC - 1))
                nc.scalar.copy(vaug_sb[:, it, :, 0:64], pv)

            def covh_job(mc, q):
                ph = pj.tile([128, 512], f32, tag="job", name=f"ph{mc}_{q}")
                nc.tensor.matmul(ph, wce1_sb[:, mc * 128:(mc + 1) * 128],
                                 covT_sb[:, q * 512:(q + 1) * 512],
                                 start=True, stop=True)
                nc.scalar.activation(
                    out=hidc_sb[:, mc, q * 512:(q + 1) * 512],
                    in_=ph, func=AFT.Silu, bias=bce1_sb[:, mc:mc + 1], scale=1.0)

            def covb_job(jt4):
                pc = pj.tile([128, 512], f32, tag="job", name=f"pc{jt4}")
                for k in range(4):
                    jt = jt4 * 4 + k
                    for mc in range(2):
                        nc.tensor.matmul(pc[:, k * 4:k * 4 + 4],
                                         hidc_sb[:, mc, jt * 128:(jt + 1) * 128],
                                         wce2_sb[:, mc * 4:(mc + 1) * 4],
                                         start=(mc == 0), stop=(mc == 1))
                for k in range(4):
                    jt = jt4 * 4 + k
                    nc.vector.tensor_add(bias_sb[:, jt, :], pc[:, k * 4:k * 4 + 4],
                                         bce2_sb)

            def gate_job():
                pg = pj.tile([128, 512], f32, tag="job", name="pg")
                for mc in range(2):
                    for dc in range(DC):
                        nc.tensor.matmul(pg[:, mc:mc + 1],
                                         wfg1_sb[:, dc, mc * 128:(mc + 1) * 128],
                                         pooled_bf[:, dc:dc + 1],
                                         start=(dc == 0), stop=(dc == DC - 1))
                for mc in range(2):
                    nc.scalar.activation(out=hidg_sb[:, mc:mc + 1],
                                         in_=pg[:, mc:mc + 1], func=AFT.Silu,
                                         bias=bfg1_sb[:, mc:mc + 1], scale=1.0 / n)
                pgp = pj.tile([128, 512], f32, tag="job", name="pgp")
                for mc in range(2):
                    nc.tensor.matmul(pgp[0:1, 0:1], hidg_sb[:, mc:mc + 1],
                                     wfg2_sb[:, mc:mc + 1],
                                     start=(mc == 0), stop=(mc == 1))
                # sigmoid(z) = 1/(1+exp(-z)) via Exp so the Sigmoid act
                # table is never loaded (one fewer table swap before the
                # exp stream); bias enters with negated sign.
                emz = consts.tile([1, 1], f32)
                nbfg2 = consts.tile([1, 1], f32)
                nc.vector.tensor_scalar_mul(out=nbfg2, in0=bfg2_sb, scalar1=-1.0)
                nc.scalar.activation(out=emz, in_=pgp[0:1, 0:1], func=AFT.Exp,
                                     bias=nbfg2, scale=-1.0)
                op1 = consts.tile([1, 1], f32)
                nc.vector.tensor_scalar_add(out=op1, in0=emz, scalar1=1.0)
                nc.vector.reciprocal(out=g_sb, in_=op1)
                pgb = pj.tile([128, 512], f32, tag="job", name="pgb")
                nc.tensor.matmul(pgb[:, 0:1], ones_f, g_sb, start=True, stop=True)
                nc.vector.tensor_copy(gb_sb, pgb[:, 0:1])
                nc.vector.tensor_scalar_mul(out=bias_sb[:, :, :],
                                            in0=bias_sb[:, :, :], scalar1=gb_sb)
                nc.vector.tensor_scalar(out=bias_dve[:, :, :], in0=bias_sb[:, :, :],
                                        scalar1=SCHRAUD_A, scalar2=SCHRAUD_B,
                                        op0=ALU.mult, op1=ALU.add)

            def oproj_job(it, half):
                py = pj.tile([128, 512], f32, tag="job", name=f"py{it}_{half}")
                for pt in range(2):
                    nc.tensor.matmul(py, attn_sb[:, pt, it * 128:(it + 1) * 128],
                                     wo_sb[:, pt, half * 512:(half + 1) * 512],
                                     start=(pt == 0), stop=(pt == 1))
                y_sb = yp.tile([128, 512], f32, tag="y", name=f"y{it}_{half}")
                nc.scalar.copy(y_sb, py)
                nc.sync.dma_start(out=OUT[it * 128:(it + 1) * 128,
                                          half * 512:(half + 1) * 512], in_=y_sb)

            # Jobs popped DURING the stream (emitted before S(jt+1)/AV(jt)).
            # Ordering rule: every producer must be EMITTED before its
            # first consumer (program-order read-before-write is a race):
            # v_job(it) before AV(jt=it) of block 0, K0 chunk ic before
            # S(4*ic) of block 0, Q0ic1 before block 1, etc. Pops are
            # paired early so v/K stay ahead of the consuming iteration.
            jobs = []
            jobs += [lambda it=it: v_job(it) for it in range(16)]  # dbl-popped
            jobs.append(lambda: qk_job(0, 1))
            jobs.append(lambda: qk_job(3, 0))
            jobs.append(lambda: qk_job(3, 1))
            jobs.append(lambda: qk_job(3, 2))
            jobs.append(lambda: qk_job(3, 3))
            jobs.append(lambda: qk_job(0, 2))
            jobs.append(lambda: qk_job(1, 0))
            jobs.append(lambda: qk_job(0, 3))
            # block 1 onward
            jobs += [lambda ic=ic: qk_job(1, ic) for ic in range(1, 4)]
            jobs.reverse()   # pop() from the end

            def s_tiles(p, ic, jt):
                out = []
                js = slice(jt * 128, (jt + 1) * 128)
                for hh in range(2):
                    lo = hh * 64
                    ps_ = pss.tile([128, IC_W], f32, tag="s",
                                   name=f"s{p}_{ic}_{jt}_{hh}")
                    nc.tensor.matmul(ps_, ktp_sb[lo:lo + 64, p, js],
                                     qt_sb[lo:lo + 64, p,
                                           ic * IC_W:(ic + 1) * IC_W],
                                     start=True, stop=True)
                    out.append(ps_)
                return out

            # pre-stream: K0 ic0 + Q0 ic0 so block 1 can start, then the
            # work that feeds bias_sb (everything the first exp needs MUST
            # precede the first AV in the in-order PE queue, or it
            # deadlocks behind it), padded with early jobs.
            # pre-stream: the minimum for block 0 + the bias_sb chain
            # (everything the first exp needs MUST precede the first AV
            # in the in-order PE queue, or it deadlocks behind it).
            qk_job(2, 0)
            qk_job(0, 0)
            for q in range(4):
                covh_job(0, q)
                covh_job(1, q)
            for j in range(4):
                covb_job(j)
            # late pooled partials + final reduction, then the qk chunks
            # that fill the PE while pooled/gate resolve
            for dc in range(4, DC):
                nc.scalar.activation(out=trash, in_=xts[:, 1, dc, :],
                                     func=AFT.Copy, scale=1.0,
                                     accum_out=pooled4[:, dc, 1:2])
            for dc in range(DC):
                nc.vector.reduce_sum(pooled_sb[:, dc:dc + 1], pooled4[:, dc, :],
                                     axis=mybir.AxisListType.X)
            nc.vector.tensor_copy(pooled_bf, pooled_sb)
            qk_job(2, 1)
            qk_job(2, 2)
            qk_job(2, 3)
            gate_job()

            blocks = [(p, ic) for p in range(2) for ic in range(NI)]
            for bi, (p, ic) in enumerate(blocks):
                po = [pop.tile([65, IC_W], f32, tag="o",
                               name=f"po{p}_{ic}_{i}") for i in range(2)]
                pend = s_tiles(p, ic, 0)
                for jt in range(NJ):
                    es = []
                    for hh in range(2):
                        h = 2 * p + hh
                        e = ep.tile([128, IC_W], bf16, tag="e",
                                    name=f"e{p}_{ic}_{jt}_{hh}")
                        if _dve_tile(jt, hh):
                            nc.vector.tensor_scalar(
                                out=e.bitcast(i16), in0=pend[hh],
                                scalar1=SCHRAUD_A * scale,
                                scalar2=bias_dve[:, jt, h:h + 1],
                                op0=ALU.mult, op1=ALU.add)
                        else:
                            nc.scalar.activation(out=e, in_=pend[hh],
                                                 func=AFT.Exp,
                                                 bias=bias_sb[:, jt, h:h + 1],
                                                 scale=scale)
                        es.append(e)
                    # jobs go BEFORE S(jt+1)/AV(jt): producers stay ahead
                    # of their consumers and the PE queue head stays
                    # runnable. Double-pop early in block 0 so v/K jobs
                    # outrun the AV/S that consume them.
                    npop = 2 if (bi == 0 and jt < 8) else 1
                    for _ in range(npop):
                        if jobs:
                            jobs.pop()()
                    if jt + 1 < NJ:
                        pend = s_tiles(p, ic, jt + 1)
                    st, sp = (jt == 0), (jt == NJ - 1)
                    for hh in range(2):
                        h = 2 * p + hh
                        nc.tensor.matmul(po[hh], vaug_sb[:, jt, h, :], es[hh],
                                         start=st, stop=sp)
                # normalize: O^T rows 0..63, denominator row 64
                osl = slice(ic * IC_W, (ic + 1) * IC_W)
                for hh in range(2):
                    lo = hh * 64
                    dd = rp.tile([1, IC_W], f32, tag="dd", name=f"dd{p}_{ic}_{hh}")
                    nc.scalar.copy(dd, po[hh][64:65, :])
                    rr = rp.tile([1, IC_W], f32, tag="rr", name=f"rr{p}_{ic}_{hh}")
                    nc.vector.reciprocal_approx_fast(out=rr, in_=dd)
                    recb = rp.tile([64, IC_W], f32, tag="recb",
                                   name=f"recb{p}_{ic}_{hh}")
                    nc.gpsimd.partition_broadcast(recb, rr)
                    nc.vector.tensor_mul(attn_sb[lo:lo + 64, p, osl],
                                         po[hh][0:64, :], recb)
                if p == 1 and ic < NI - 1:
                    for it in range(ic * 4, ic * 4 + 4):
                        jobs.append(lambda it=it, h=1: oproj_job(it, h))
                        jobs.append(lambda it=it, h=0: oproj_job(it, h))
            while jobs:
                jobs.pop()()

        if dbg:
            nc.sync.dma_start(out=DBG_Q, in_=qt_sb)
            nc.sync.dma_start(out=DBG_K, in_=ktp_sb)
            nc.sync.dma_start(out=DBG_V, in_=vaug_sb)
            nc.sync.dma_start(out=DBG_B, in_=bias_sb)
            nc.sync.dma_start(out=DBG_A, in_=attn_sb)

        # ---- tail: last block's out-projection, DMA direct from PSUM ----
        with tc.tile_pool(name="psy", bufs=2, space="PSUM") as psy:
            for it in range(NJ - 4, NJ):
                py = psy.tile([128, D], f32, tag="y")
                for pt in range(2):
                    for half in range(2):
                        nc.tensor.matmul(
                            py[:, half * 512:(half + 1) * 512],
                            attn_sb[:, pt, it * 128:(it + 1) * 128],
                            wo_sb[:, pt, half * 512:(half + 1) * 512],
                            start=(pt == 0), stop=(pt == 1))
                y_sb = yp.tile([128, D], f32, tag="yt", name=f"ysb{it}")
                if it % 2 == 0:
                    nc.vector.tensor_copy(y_sb, py)
                else:
                    nc.scalar.copy(y_sb, py)
                eng = nc.sync if it % 2 == 0 else nc.scalar
                eng.dma_start(out=OUT[it * 128:(it + 1) * 128, :], in_=y_sb)

    nc.compile()
    return nc


def make_in_maps(x, coverage, w_qkv, w_out, b_out, w_ce1, b_ce1, w_ce2, b_ce2,
                 w_fg1, b_fg1, w_fg2, b_fg2, n=N):
    f = np.float32
    DC = D // 128
    x = np.asarray(x, f)
    coverage = np.asarray(coverage, f)
    w_qkv = np.asarray(w_qkv, f)
    w_out = np.asarray(w_out, f)

    def pmajor(a, inner):
        blocks = a.shape[0] // 128
        return np.ascontiguousarray(
            a.reshape(blocks, 128, inner).transpose(1, 0, 2))

    smf = np.concatenate([
        np.asarray(b_ce1, f).reshape(2, 128).T,
        np.tile(np.asarray(b_ce2, f).reshape(1, 16)[:, 0:4], (128, 1)) * 0,  # per-core below
        np.asarray(b_fg1, f).reshape(2, 128).T,
        np.asarray(w_fg2, f).reshape(2, 128).T,
        np.full((128, 1), np.asarray(b_fg2, f).reshape(()), f),
    ], axis=1)

    in_maps = []
    for c in range(NCORES):
        b, hg = divmod(c, 4)
        cs, ce = hg * 256, (hg + 1) * 256
        wq = w_qkv[:, 0 * D + cs:0 * D + ce]
        wk = w_qkv[:, 1 * D + cs:1 * D + ce]
        wv = w_qkv[:, 2 * D + cs:2 * D + ce]
        wqk4 = np.concatenate([wq, wk], axis=1)
        wqk4 = wqk4.reshape(DC, 128, 4, 128).transpose(1, 2, 0, 3)
        xt = x[b].T.reshape(DC, 128, 2, 1024).transpose(1, 2, 0, 3)
        smf_c = smf.copy()
        smf_c[:, 2:6] = np.tile(
            np.asarray(b_ce2, f)[4 * hg:4 * hg + 4][None, :], (128, 1))
        m = {
            "xT": _bf16(xt),
            "wqk": _bf16(wqk4),
            "wv": _bf16(pmajor(wv, 256)),
            "wo": _bf16(pmajor(w_out[cs:ce, :], D)),
            "cvw": _bf16(np.concatenate([coverage[b, :, 0],
                                         np.asarray(w_ce1, f).reshape(-1)])[None, :]),
            "wce2": _bf16(
                np.asarray(w_ce2, f)[:, 4 * hg:4 * hg + 4].reshape(2, 128, 4)
                .transpose(1, 0, 2).reshape(128, 8)),
            "smf": smf_c,
            "wfg1": _bf16(pmajor(np.asarray(w_fg1, f), 256)),
        }
        in_maps.append(m)
    return in_maps


def kernel(**inputs):
    from concourse.bass_utils import run_bass_kernel_spmd
    if "nc" not in _COMPILED:
        _COMPILED["nc"] = build(N)
    nc = _COMPILED["nc"]
    in_maps = make_in_maps(**inputs)
    res = run_bass_kernel_spmd(nc, in_maps, core_ids=list(range(NCORES)))
    outs = [np.asarray(res.results[c]["out"], dtype=np.float32)
            for c in range(NCORES)]
    b_out = np.asarray(inputs["b_out"], np.float32)
    full = np.stack([
        outs[0] + outs[1] + outs[2] + outs[3] + b_out[None, :],
        outs[4] + outs[5] + outs[6] + outs[7] + b_out[None, :],
    ]).astype(np.float32)
    return full


# revision 40
# speedup vs baseline: 1.0082x; 1.0082x over previous
"""AdaptiveCoverageAttention TRN2 kernel: 8-way (batch x head-group) sharded.

Sharding: core c in 0..7 -> batch b = c//4, head-group hg = c%4 (4 heads each).
Each core computes its 4 heads' attention + its partial output projection;
the host sums the 4 partials per batch (and adds b_out). No collectives.

v5: PE-roofline oriented (PE ~393k cycles/core @2.4GHz = 164us).
- IC_W=512: every stream PSUM tile is one bank. pss bufs=3 gives the
  S->exp->S chain 1.5 iterations of slack; po bufs=4 gives normalize a
  whole block of slack. Job pool (1 bank) hosts all projection/MLP/out-proj
  matmuls INTERLEAVED into the stream so the in-order PE queue never
  drains (keeps the PE DVFS p-state at 2.4GHz).
- exp tiles [128,512]: hh1/jt-odd quarter runs on VectorE via Schraudolph
  bf16 (int16 convert + bitcast, mean-centered C=-7.37, ~+7e-3 rel err),
  rest on ScalarE.
- Pooled sums for the gate MLP: half on DVE reduce, half via ScalarE
  activation accum_out, so the gate (which gates the first exp) is ready
  ~23us in.
- Host pre-packs everything partition-major; ~17 large DMAs on the two
  HW DGE rings, small consts packed into 3 DMAs.
- Normalize per (p,ic): dd copy + reciprocal on DVE (recip misreads
  partition-offset PSUM APs, so copy to partition 0 first), broadcast on
  GpSimd, mul on DVE.
- Out-projection runs as jobs after both pairs of an i-range normalize;
  last block's 4 tiles in a short tail.
"""
import os as _os
import sys

sys.path.insert(0, "/opt/trn_rl_repo")

import numpy as np

B, N, D, H = 2, 2048, 1024, 16
HD = D // H            # 64
NCORES = 8
IC_W = 512

_COMPILED = {}

SCHRAUD_A = float(128.0 * np.log2(np.e))
SCHRAUD_B = float(127.0 * 128.0 - 7.37)
_DVE_OFF = bool(int(_os.environ.get("KDVE_OFF", "0")))


def _bf16(x):
    import ml_dtypes
    return np.ascontiguousarray(np.asarray(x, np.float32)).astype(ml_dtypes.bfloat16)


def _dve_tile(jt, hh):
    """Which exp tiles run on VectorE (Schraudolph). 50% of tiles: the
    hh1 tiles, so the hh0 PSUM-bank chain runs through ScalarE and the
    hh1 chain through VectorE, fully decoupled."""
    if _DVE_OFF:
        return False
    return hh == 1


def build(n=N):
    import contextlib

    import concourse.bacc as bacc
    import concourse.tile as tile
    from concourse import mybir

    f32 = mybir.dt.float32
    bf16 = mybir.dt.bfloat16
    i16 = mybir.dt.int16
    AFT = mybir.ActivationFunctionType
    ALU = mybir.AluOpType

    NJ = n // 128          # 16 j-tiles
    NI = n // 512          # 4 i-chunks of 512 (also = stream blocks/pair)
    DC = D // 128          # 8 contraction chunks
    scale = float(HD) ** -0.5

    nc = bacc.Bacc("TRN2", target_bir_lowering=False, debug=False,
                   num_devices=NCORES)

    dram = lambda name, shape, dt, kind: nc.dram_tensor(name, shape, dt, kind=kind).ap()
    XT = dram("xT", [128, 2, DC, 1024], bf16, "ExternalInput")     # (p, jc2, dc, tok)
    WQK = dram("wqk", [128, 4, DC, 128], bf16, "ExternalInput")    # (p, cb, dc, col)
    WV = dram("wv", [128, DC, 256], bf16, "ExternalInput")
    WO = dram("wo", [128, 2, D], bf16, "ExternalInput")
    CVW = dram("cvw", [1, n + 256], bf16, "ExternalInput")         # covT | wce1
    WCE2 = dram("wce2", [128, 8], bf16, "ExternalInput")
    SMF = dram("smf", [128, 11], f32, "ExternalInput")  # bce1|bce2|bfg1|wfg2|bfg2
    WFG1 = dram("wfg1", [128, DC, 256], bf16, "ExternalInput")
    OUT = dram("out", [n, D], f32, "ExternalOutput")
    dbg = bool(int(_os.environ.get("KDBG", "0")))
    if dbg:
        DBG_Q = dram("dbg_q", [128, 2, n], bf16, "ExternalOutput")
        DBG_K = dram("dbg_k", [128, 2, n], bf16, "ExternalOutput")
        DBG_V = dram("dbg_v", [128, NJ, 4, 65], bf16, "ExternalOutput")
        DBG_B = dram("dbg_b", [128, NJ, 4], f32, "ExternalOutput")
        DBG_A = dram("dbg_a", [128, 2, n], bf16, "ExternalOutput")

    with tile.TileContext(nc) as tc, contextlib.ExitStack() as ctx:
        consts = ctx.enter_context(tc.tile_pool(name="consts", bufs=1))
        xtp = ctx.enter_context(tc.tile_pool(name="xtp", bufs=1))
        qkv = ctx.enter_context(tc.tile_pool(name="qkv", bufs=1))
        big2 = ctx.enter_context(tc.tile_pool(name="big2", bufs=1))
        ep = ctx.enter_context(tc.tile_pool(name="ep", bufs=8))
        rp = ctx.enter_context(tc.tile_pool(name="rp", bufs=4))
        yp = ctx.enter_context(tc.tile_pool(name="yp", bufs=6))

        wqk_sb = consts.tile([128, 4, DC, 128], bf16)
        wv_sb = consts.tile([128, DC, 256], bf16)
        wo_sb = consts.tile([128, 2, D], bf16)
        cvw_sb = consts.tile([1, n + 256], bf16)
        covT_sb = cvw_sb[:, 0:n]
        wce1_sb = cvw_sb[:, n:n + 256]
        wce2_sb = consts.tile([128, 8], bf16)
        smf_sb = consts.tile([128, 11], f32)
        bce1_sb = smf_sb[:, 0:2]
        bce2_sb = smf_sb[:, 2:6]
        bfg1_sb = smf_sb[:, 6:8]
        wfg2_sb = smf_sb[:, 8:10]
        bfg2_sb = smf_sb[0:1, 10:11]
        wfg1_sb = consts.tile([128, DC, 256], bf16)
        xts = xtp.tile([128, 2, DC, 1024], bf16, name="xts")

        ones_f = consts.tile([1, 128], f32)
        pooled4 = consts.tile([128, DC, 2], f32)
        pooled_sb = consts.tile([128, DC], f32)
        pooled_bf = consts.tile([128, DC], bf16)
        trash = consts.tile([128, 1024], f32)
        hidg_sb = consts.tile([128, 2], f32)
        g_sb = consts.tile([1, 1], f32)
        gb_sb = consts.tile([128, 1], f32)
        bias_sb = consts.tile([128, NJ, 4], f32)
        bias_dve = consts.tile([128, NJ, 4], f32)

        # ---- DMA schedule: two HW rings, time-critical first. The first
        # 512 tokens of jc0 go in their own (smaller) DMAs so the first
        # qk chunk can start ~2us earlier. Pooled partial sums ride along:
        # DVE reduces the lo halves, ScalarE accum_out the hi halves
        # (emitted between this ring's own DMA issues).
        nc.sync.dma_start(out=wqk_sb[:, 2], in_=WQK[:, 2])       # K0
        nc.sync.dma_start(out=xts[:, 0, 0:4, 0:512], in_=XT[:, 0, 0:4, 0:512])
        nc.sync.dma_start(out=xts[:, 0, 0:4, 512:1024],
                          in_=XT[:, 0, 0:4, 512:1024])
        nc.sync.dma_start(out=wqk_sb[:, 0], in_=WQK[:, 0])       # Q0
        nc.sync.dma_start(out=xts[:, 1, 0:4], in_=XT[:, 1, 0:4])
        nc.sync.dma_start(out=wfg1_sb, in_=WFG1)
        nc.sync.dma_start(out=wo_sb, in_=WO)

        nc.scalar.dma_start(out=xts[:, 0, 4:8, 0:512], in_=XT[:, 0, 4:8, 0:512])
        nc.scalar.dma_start(out=xts[:, 0, 4:8, 512:1024],
                            in_=XT[:, 0, 4:8, 512:1024])
        nc.scalar.dma_start(out=cvw_sb, in_=CVW)
        nc.scalar.dma_start(out=smf_sb, in_=SMF)
        nc.scalar.dma_start(out=wce2_sb, in_=WCE2)
        nc.scalar.dma_start(out=wv_sb, in_=WV)
        nc.scalar.dma_start(out=wqk_sb[:, 3], in_=WQK[:, 3])     # K1
        nc.scalar.dma_start(out=wqk_sb[:, 1], in_=WQK[:, 1])     # Q1
        nc.scalar.dma_start(out=xts[:, 1, 4:8], in_=XT[:, 1, 4:8])

        nc.vector.memset(ones_f, 1.0)
        # 12 of the 16 pooled partials on DVE (jc1-hi comes via ScalarE
        # accum_out, emitted later in the pre-stream so it doesn't block
        # the cov-MLP silus behind late DMA data)
        for jc2 in range(2):
            for dc in range(4):
                nc.vector.reduce_sum(pooled4[:, dc, jc2:jc2 + 1],
                                     xts[:, jc2, dc, :],
                                     axis=mybir.AxisListType.X)
        for dc in range(4, DC):
            nc.vector.reduce_sum(pooled4[:, dc, 0:1], xts[:, 0, dc, :],
                                 axis=mybir.AxisListType.X)

        qt_sb = qkv.tile([128, 2, n], bf16)
        ktp_sb = qkv.tile([128, 2, n], bf16)
        vaug_sb = qkv.tile([128, NJ, 4, 65], bf16)
        nc.vector.memset(vaug_sb, 1.0)
        hidc_sb = big2.tile([128, 2, n], bf16, tag="big", name="hidc")
        attn_sb = big2.tile([128, 2, n], bf16, tag="big", name="attn")

        # ================= stream with interleaved jobs =================
        with tc.tile_pool(name="pss", bufs=4, space="PSUM") as pss, \
             tc.tile_pool(name="pop", bufs=3, space="PSUM") as pop, \
             tc.tile_pool(name="pj", bufs=1, space="PSUM") as pj:

            cp_i = [0]

            def cp_eng():
                cp_i[0] += 1
                return nc.vector if cp_i[0] % 2 == 0 else nc.scalar

            def copy(eng, dst, src):
                if eng is nc.scalar:
                    eng.copy(dst, src)
                else:
                    eng.tensor_copy(dst, src)

            def qk_job(cb, ic):
                pq = pj.tile([128, 512], f32, tag="job", name=f"pq{cb}_{ic}")
                jc2, sub = ic // 2, (ic % 2) * 512
                for dc in range(DC):
                    nc.tensor.matmul(pq, wqk_sb[:, cb, dc, :],
                                     xts[:, jc2, dc, sub:sub + 512],
                                     start=(dc == 0), stop=(dc == DC - 1))
                dst = (ktp_sb[:, cb - 2, ic * 512:(ic + 1) * 512] if cb >= 2
                       else qt_sb[:, cb, ic * 512:(ic + 1) * 512])
                copy(cp_eng(), dst, pq)

            def v_job(it):
                pv = pj.tile([128, 4, 64], f32, tag="job", name=f"pv{it}")
                jc2, col = it // 8, (it % 8) * 128
                for dc in range(DC):
                    nc.tensor.matmul(pv, xts[:, jc2, dc, col:col + 128],
                                     wv_sb[:, dc, :],
                                     start=(dc == 0), stop=(dc == DC - 1))
                nc.scalar.copy(vaug_sb[:, it, :, 0:64], pv)

            def covh_job(mc, q):
                ph = pj.tile([128, 512], f32, tag="job", name=f"ph{mc}_{q}")
                nc.tensor.matmul(ph, wce1_sb[:, mc * 128:(mc + 1) * 128],
                                 covT_sb[:, q * 512:(q + 1) * 512],
                                 start=True, stop=True)
                nc.scalar.activation(
                    out=hidc_sb[:, mc, q * 512:(q + 1) * 512],
                    in_=ph, func=AFT.Silu, bias=bce1_sb[:, mc:mc + 1], scale=1.0)

            def covb_job(jt4):
                pc = pj.tile([128, 512], f32, tag="job", name=f"pc{jt4}")
                for k in range(4):
                    jt = jt4 * 4 + k
                    for mc in range(2):
                        nc.tensor.matmul(pc[:, k * 4:k * 4 + 4],
                                         hidc_sb[:, mc, jt * 128:(jt + 1) * 128],
                                         wce2_sb[:, mc * 4:(mc + 1) * 4],
                                         start=(mc == 0), stop=(mc == 1))
                for k in range(4):
                    jt = jt4 * 4 + k
                    nc.vector.tensor_add(bias_sb[:, jt, :], pc[:, k * 4:k * 4 + 4],
                                         bce2_sb)

            def gate_job():
                pg = pj.tile([128, 512], f32, tag="job", name="pg")
                for mc in range(2):
                    for dc in range(DC):
                        nc.tensor.matmul(pg[:, mc:mc + 1],
                                         wfg1_sb[:, dc, mc * 128:(mc + 1) * 128],
                                         pooled_bf[:, dc:dc + 1],
                                         start=(dc == 0), stop=(dc == DC - 1))
                for mc in range(2):
                    nc.scalar.activation(out=hidg_sb[:, mc:mc + 1],
                                         in_=pg[:, mc:mc + 1], func=AFT.Silu,
                                         bias=bfg1_sb[:, mc:mc + 1], scale=1.0 / n)
                pgp = pj.tile([128, 512], f32, tag="job", name="pgp")
                for mc in range(2):
                    nc.tensor.matmul(pgp[0:1, 0:1], hidg_sb[:, mc:mc + 1],
                                     wfg2_sb[:, mc:mc + 1],
                                     start=(mc == 0), stop=(mc == 1))
                # sigmoid(z) = 1/(1+exp(-z)) via Exp so the Sigmoid act
                # table is never loaded (one fewer table swap before the
                # exp stream); bias enters with negated sign.
                emz = consts.tile([1, 1], f32)
                nbfg2 = consts.tile([1, 1], f32)
                nc.vector.tensor_scalar_mul(out=nbfg2, in0=bfg2_sb, scalar1=-1.0)
                nc.scalar.activation(out=emz, in_=pgp[0:1, 0:1], func=AFT.Exp,
                                     bias=nbfg2, scale=-1.0)
                op1 = consts.tile([1, 1], f32)
                nc.vector.tensor_scalar_add(out=op1, in0=emz, scalar1=1.0)
                nc.vector.reciprocal(out=g_sb, in_=op1)
                pgb = pj.tile([128, 512], f32, tag="job", name="pgb")
                nc.tensor.matmul(pgb[:, 0:1], ones_f, g_sb, start=True, stop=True)
                nc.vector.tensor_copy(gb_sb, pgb[:, 0:1])
                nc.vector.tensor_scalar_mul(out=bias_sb[:, :, :],
                                            in0=bias_sb[:, :, :], scalar1=gb_sb)
                nc.vector.tensor_scalar(out=bias_dve[:, :, :], in0=bias_sb[:, :, :],
                                        scalar1=SCHRAUD_A, scalar2=SCHRAUD_B,
                                        op0=ALU.mult, op1=ALU.add)

            def oproj_job(it, half):
                py = pj.tile([128, 512], f32, tag="job", name=f"py{it}_{half}")
                for pt in range(2):
                    nc.tensor.matmul(py, attn_sb[:, pt, it * 128:(it + 1) * 128],
                                     wo_sb[:, pt, half * 512:(half + 1) * 512],
                                     start=(pt == 0), stop=(pt == 1))
                y_sb = yp.tile([128, 512], f32, tag="y", name=f"y{it}_{half}")
                nc.scalar.copy(y_sb, py)
                nc.sync.dma_start(out=OUT[it * 128:(it + 1) * 128,
                                          half * 512:(half + 1) * 512], in_=y_sb)

            # Jobs popped DURING the stream (emitted before S(jt+1)/AV(jt)).
            # Ordering rule: every producer must be EMITTED before its
            # first consumer (program-order read-before-write is a race):
            # v_job(it) before AV(jt=it) of block 0, K0 chunk ic before
            # S(4*ic) of block 0, Q0ic1 before block 1, etc. Pops are
            # paired early so v/K stay ahead of the consuming iteration.
            jobs = []
            jobs += [lambda it=it: v_job(it) for it in range(16)]  # dbl-popped
            jobs.append(lambda: qk_job(0, 1))
            jobs.append(lambda: qk_job(3, 0))
            jobs.append(lambda: qk_job(3, 1))
            jobs.append(lambda: qk_job(3, 2))
            jobs.append(lambda: qk_job(3, 3))
            jobs.append(lambda: qk_job(0, 2))
            jobs.append(lambda: qk_job(1, 0))
            jobs.append(lambda: qk_job(0, 3))
            # block 1 onward
            jobs += [lambda ic=ic: qk_job(1, ic) for ic in range(1, 4)]
            jobs.reverse()   # pop() from the end

            def s_tiles(p, ic, jt):
                out = []
                js = slice(jt * 128, (jt + 1) * 128)
                for hh in range(2):
                    lo = hh * 64
                    ps_ = pss.tile([128, IC_W], f32, tag="s",
                                   name=f"s{p}_{ic}_{jt}_{hh}")
                    nc.tensor.matmul(ps_, ktp_sb[lo:lo + 64, p, js],
                                     qt_sb[lo:lo + 64, p,
                                           ic * IC_W:(ic + 1) * IC_W],
                                     start=True, stop=True)
                    out.append(ps_)
                return out

            # pre-stream: K0 ic0 + Q0 ic0 so block 1 can start, then the
            # work that feeds bias_sb (everything the first exp needs MUST
            # precede the first AV in the in-order PE queue, or it
            # deadlocks behind it), padded with early jobs.
            # pre-stream: the minimum for block 0 + the bias_sb chain
            # (everything the first exp needs MUST precede the first AV
            # in the in-order PE queue, or it deadlocks behind it).
            qk_job(2, 0)
            qk_job(0, 0)
            for q in range(4):
                covh_job(0, q)
                covh_job(1, q)
            for j in range(4):
                covb_job(j)
            # late pooled partials + final reduction, then the qk chunks
            # that fill the PE while pooled/gate resolve
            for dc in range(4, DC):
                nc.scalar.activation(out=trash, in_=xts[:, 1, dc, :],
                                     func=AFT.Copy, scale=1.0,
                                     accum_out=pooled4[:, dc, 1:2])
            for dc in range(DC):
                nc.vector.reduce_sum(pooled_sb[:, dc:dc + 1], pooled4[:, dc, :],
                                     axis=mybir.AxisListType.X)
            nc.vector.tensor_copy(pooled_bf, pooled_sb)
            qk_job(2, 1)
            qk_job(2, 2)
            qk_job(2, 3)
            gate_job()

            blocks = [(p, ic) for p in range(2) for ic in range(NI)]
            for bi, (p, ic) in enumerate(blocks):
                po = [pop.tile([65, IC_W], f32, tag="o",
                               name=f"po{p}_{ic}_{i}") for i in range(2)]
                pend = s_tiles(p, ic, 0)
                for jt in range(NJ):
                    es = []
                    for hh in range(2):
                        h = 2 * p + hh
                        e = ep.tile([128, IC_W], bf16, tag="e",
                                    name=f"e{p}_{ic}_{jt}_{hh}")
                        if _dve_tile(jt, hh):
                            nc.vector.tensor_scalar(
                                out=e.bitcast(i16), in0=pend[hh],
                                scalar1=SCHRAUD_A * scale,
                                scalar2=bias_dve[:, jt, h:h + 1],
                                op0=ALU.mult, op1=ALU.add)
                        else:
                            nc.scalar.activation(out=e, in_=pend[hh],
                                                 func=AFT.Exp,
                                                 bias=bias_sb[:, jt, h:h + 1],
                                                 scale=scale)
                        es.append(e)
                    # jobs go BEFORE S(jt+1)/AV(jt): producers stay ahead
                    # of their consumers and the PE queue head stays
                    # runnable. Double-pop early in block 0 so v/K jobs
                    # outrun the AV/S that consume them.
                    npop = 2 if (bi == 0 and jt < 8) else 1
                    for _ in range(npop):
                        if jobs:
                            jobs.pop()()
                    if jt + 1 < NJ:
                        pend = s_tiles(p, ic, jt + 1)
                    st, sp = (jt == 0), (jt == NJ - 1)
                    for hh in range(2):
                        h = 2 * p + hh
                        nc.tensor.matmul(po[hh], vaug_sb[:, jt, h, :], es[hh],
                                         start=st, stop=sp)
                # normalize: O^T rows 0..63, denominator row 64
                osl = slice(ic * IC_W, (ic + 1) * IC_W)
                for hh in range(2):
                    lo = hh * 64
                    dd = rp.tile([1, IC_W], f32, tag="dd", name=f"dd{p}_{ic}_{hh}")
                    nc.scalar.copy(dd, po[hh][64:65, :])
                    rr = rp.tile([1, IC_W], f32, tag="rr", name=f"rr{p}_{ic}_{hh}")
                    nc.vector.reciprocal_approx_fast(out=rr, in_=dd)
                    recb = rp.tile([64, IC_W], f32, tag="recb",
                                   name=f"recb{p}_{ic}_{hh}")
                    nc.gpsimd.partition_broadcast(recb, rr)
                    nc.vector.tensor_mul(attn_sb[lo:lo + 64, p, osl],
                                         po[hh][0:64, :], recb)
                if p == 1 and ic < NI - 1:
                    for it in range(ic * 4, ic * 4 + 4):
                        jobs.append(lambda it=it, h=1: oproj_job(it, h))
                        jobs.append(lambda it=it, h=0: oproj_job(it, h))
            while jobs:
                jobs.pop()()

        if dbg:
            nc.sync.dma_start(out=DBG_Q, in_=qt_sb)
            nc.sync.dma_start(out=DBG_K, in_=ktp_sb)
            nc.sync.dma_start(out=DBG_V, in_=vaug_sb)
            nc.sync.dma_start(out=DBG_B, in_=bias_sb)
            nc.sync.dma_start(out=DBG_A, in_=attn_sb)

        # ---- tail: last block's out-projection, DMA direct from PSUM ----
        with tc.tile_pool(name="psy", bufs=2, space="PSUM") as psy:
            for it in range(NJ - 4, NJ):
                py = psy.tile([128, D], f32, tag="y")
                for pt in range(2):
                    for half in range(2):
                        nc.tensor.matmul(
                            py[:, half * 512:(half + 1) * 512],
                            attn_sb[:, pt, it * 128:(it + 1) * 128],
                            wo_sb[:, pt, half * 512:(half + 1) * 512],
                            start=(pt == 0), stop=(pt == 1))
                y_sb = yp.tile([128, D], f32, tag="yt", name=f"ysb{it}")
                if it % 2 == 0:
                    nc.vector.tensor_copy(y_sb, py)
                else:
                    nc.scalar.copy(y_sb, py)
                eng = nc.sync if it % 2 == 0 else nc.scalar
                eng.dma_start(out=OUT[it * 128:(it + 1) * 128, :], in_=y_sb)

    nc.compile()
    return nc


def make_in_maps(x, coverage, w_qkv, w_out, b_out, w_ce1, b_ce1, w_ce2, b_ce2,
                 w_fg1, b_fg1, w_fg2, b_fg2, n=N):
    f = np.float32
    DC = D // 128
    x = np.asarray(x, f)
    coverage = np.asarray(coverage, f)
    w_qkv = np.asarray(w_qkv, f)
    w_out = np.asarray(w_out, f)

    def pmajor(a, inner):
        blocks = a.shape[0] // 128
        return np.ascontiguousarray(
            a.reshape(blocks, 128, inner).transpose(1, 0, 2))

    smf = np.concatenate([
        np.asarray(b_ce1, f).reshape(2, 128).T,
        np.tile(np.asarray(b_ce2, f).reshape(1, 16)[:, 0:4], (128, 1)) * 0,  # per-core below
        np.asarray(b_fg1, f).reshape(2, 128).T,
        np.asarray(w_fg2, f).reshape(2, 128).T,
        np.full((128, 1), np.asarray(b_fg2, f).reshape(()), f),
    ], axis=1)

    in_maps = []
    for c in range(NCORES):
        b, hg = divmod(c, 4)
        cs, ce = hg * 256, (hg + 1) * 256
        wq = w_qkv[:, 0 * D + cs:0 * D + ce]
        wk = w_qkv[:, 1 * D + cs:1 * D + ce]
        wv = w_qkv[:, 2 * D + cs:2 * D + ce]
        wqk4 = np.concatenate([wq, wk], axis=1)
        wqk4 = wqk4.reshape(DC, 128, 4, 128).transpose(1, 2, 0, 3)
        xt = x[b].T.reshape(DC, 128, 2, 1024).transpose(1, 2, 0, 3)
        smf_c = smf.copy()
        smf_c[:, 2:6] = np.tile(
            np.asarray(b_ce2, f)[4 * hg:4 * hg + 4][None, :], (128, 1))
        m = {
            "xT": _bf16(xt),
            "wqk": _bf16(wqk4),
            "wv": _bf16(pmajor(wv, 256)),
            "wo": _bf16(pmajor(w_out[cs:ce, :], D)),
            "cvw": _bf16(np.concatenate([coverage[b, :, 0],
                                         np.asarray(w_ce1, f).reshape(-1)])[None, :]),
            "wce2": _bf16(
                np.asarray(w_ce2, f)[:, 4 * hg:4 * hg + 4].reshape(2, 128, 4)
                .transpose(1, 0, 2).reshape(128, 8)),
            "smf": smf_c,
            "wfg1": _bf16(pmajor(np.asarray(w_fg1, f), 256)),
        }
        in_maps.append(m)
    return in_maps


def kernel(**inputs):
    from concourse.bass_utils import run_bass_kernel_spmd
    if "nc" not in _COMPILED:
        _COMPILED["nc"] = build(N)
    nc = _COMPILED["nc"]
    in_maps = make_in_maps(**inputs)
    res = run_bass_kernel_spmd(nc, in_maps, core_ids=list(range(NCORES)))
    outs = [np.asarray(res.results[c]["out"], dtype=np.float32)
            for c in range(NCORES)]
    b_out = np.asarray(inputs["b_out"], np.float32)
    full = np.stack([
        outs[0] + outs[1] + outs[2] + outs[3] + b_out[None, :],
        outs[4] + outs[5] + outs[6] + outs[7] + b_out[None, :],
    ]).astype(np.float32)
    return full
